# revision 1
# baseline (speedup 1.0000x reference)
import numpy as np

import concourse.bass as bass
import concourse.bacc as bacc
import concourse.mybir as mybir
import concourse.tile as tile
from concourse import bass_isa
from concourse.bass_utils import run_bass_kernel_spmd

F32 = mybir.dt.float32
ALU = mybir.AluOpType
AXL = mybir.AxisListType
ACTF = mybir.ActivationFunctionType

B, P, C, J = 32, 16384, 80, 50
Q, T = 128, 128           # p = t*128 + q
NB = 4                    # images per core
NCORES = 8
TC = 8                    # conf chunks per image
TCH = T // TC             # 16 t's per conf chunk
MINE_ITERS = 26

_CACHE = {}
import os
PH = int(os.environ.get('KPH', '9'))


def _ap(base, offset_elems, dims):
    """Build an AP on the same tensor as `base` ([step,count] dims after partition)."""
    return bass.AP(tensor=base.tensor, offset=base.offset + offset_elems,
                   ap=[base.ap[0]] + [list(d) for d in dims])


def build_nc():
    nc = bacc.Bacc("TRN2", target_bir_lowering=False, debug=False)
    loc = nc.dram_tensor("loc", [NB, P, 4], F32, kind="ExternalInput")
    conf = nc.dram_tensor("conf", [NB, P, C], F32, kind="ExternalInput")
    obj = nc.dram_tensor("obj", [NB, P, 2], F32, kind="ExternalInput")
    tgt = nc.dram_tensor("tgt", [NB, J, 6], F32, kind="ExternalInput")
    pri = nc.dram_tensor("pri", [P, 4], F32, kind="ExternalInput")
    out = nc.dram_tensor("out", [1, 8], F32, kind="ExternalOutput")

    eye_d = nc.inline_tensor(np.eye(128, dtype=np.float32), "eye128")
    iota80_d = nc.inline_tensor(np.arange(80, dtype=np.float32)[None, :], "iota80")
    epsj_d = nc.inline_tensor(((J - np.arange(J, dtype=np.float64)) * 2.0**-120
                               ).astype(np.float32)[None, :], "epsj")
    j2_d = nc.inline_tensor((2.0 + np.arange(J, dtype=np.float64) * 2.0**-17
                             ).astype(np.float32)[None, :], "j2col")

    with tile.TileContext(nc) as tc:
        cp = tc.alloc_tile_pool(name="const", bufs=1)
        tp = tc.alloc_tile_pool(name="tgtp", bufs=1)
        bp_ = tc.alloc_tile_pool(name="big", bufs=1)
        pmp = tc.alloc_tile_pool(name="pm", bufs=1)
        sm = tc.alloc_tile_pool(name="sm", bufs=1)
        accp = tc.alloc_tile_pool(name="acc", bufs=2)
        cfp = tc.alloc_tile_pool(name="cf", bufs=1)
        psp = tc.alloc_tile_pool(name="psum", bufs=1, space="PSUM")

        # ---------------- constants / prior-derived ----------------
        PRI = cp.tile([128, T, 4], F32, tag="pri")
        nc.sync.dma_start(out=PRI[:], in_=bass.AP(tensor=pri, offset=0,
                          ap=[[4, 128], [512, T], [1, 4]]))
        EYE = cp.tile([128, 128], F32, tag="eye")
        nc.sync.dma_start(out=EYE[:], in_=bass.AP(tensor=eye_d, offset=0,
                          ap=[[128, 128], [1, 128]]))
        ONES = cp.tile([128, 128], F32, tag="ones")
        nc.vector.memset(ONES[:], 1.0)

        IOTA80 = cp.tile([128, 80], F32, tag="io80")
        t0 = sm.tile([1, 80], F32, tag="t0a")
        nc.sync.dma_start(out=t0[:], in_=bass.AP(tensor=iota80_d, offset=0, ap=[[80, 1], [1, 80]]))
        nc.gpsimd.partition_broadcast(IOTA80[:], t0[:])
        EPSJ = cp.tile([128, J], F32, tag="epsj")
        t1 = sm.tile([1, J], F32, tag="t0b")
        nc.sync.dma_start(out=t1[:], in_=bass.AP(tensor=epsj_d, offset=0, ap=[[J, 1], [1, J]]))
        nc.gpsimd.partition_broadcast(EPSJ[:], t1[:])
        J2 = cp.tile([128, J], F32, tag="j2")
        t2 = sm.tile([1, J], F32, tag="t0c")
        nc.sync.dma_start(out=t2[:], in_=bass.AP(tensor=j2_d, offset=0, ap=[[J, 1], [1, J]]))
        nc.gpsimd.partition_broadcast(J2[:], t2[:])

        # prior-derived [128,T] planes
        PD = cp.tile([128, T, 8], F32, tag="pd")  # px1 px2 py1 py2 areap idpw idph spare
        pv = lambda k: _ap(PRI[:], k, [[4, T]])
        pd = lambda k: _ap(PD[:], k, [[8, T]])
        PCX, PCY, PW_, PH_ = pv(0), pv(1), pv(2), pv(3)
        nc.vector.scalar_tensor_tensor(out=pd(0), in0=PW_, scalar=-0.5, in1=PCX,
                                       op0=ALU.mult, op1=ALU.add)
        nc.vector.scalar_tensor_tensor(out=pd(1), in0=PW_, scalar=0.5, in1=PCX,
                                       op0=ALU.mult, op1=ALU.add)
        nc.vector.scalar_tensor_tensor(out=pd(2), in0=PH_, scalar=-0.5, in1=PCY,
                                       op0=ALU.mult, op1=ALU.add)
        nc.vector.scalar_tensor_tensor(out=pd(3), in0=PH_, scalar=0.5, in1=PCY,
                                       op0=ALU.mult, op1=ALU.add)
        tw = sm.tile([128, T], F32, tag="tw")
        th = sm.tile([128, T], F32, tag="th")
        nc.vector.tensor_tensor(out=tw[:], in0=pd(1), in1=pd(0), op=ALU.subtract)
        nc.vector.tensor_tensor(out=th[:], in0=pd(3), in1=pd(2), op=ALU.subtract)
        nc.vector.tensor_tensor(out=pd(4), in0=tw[:], in1=th[:], op=ALU.mult)
        dpw = sm.tile([128, T], F32, tag="dpw")
        nc.vector.tensor_scalar(out=dpw[:], in0=PW_, scalar1=0.1, scalar2=None, op0=ALU.mult)
        nc.vector.reciprocal(out=pd(5), in_=dpw[:])
        nc.vector.tensor_scalar(out=dpw[:], in0=PH_, scalar1=0.1, scalar2=None, op0=ALU.mult)
        nc.vector.reciprocal(out=pd(6), in_=dpw[:])
        IPW = cp.tile([128, T, 2], F32, tag="ipw")  # 1/pw, 1/ph
        nc.vector.reciprocal(out=_ap(IPW[:], 0, [[2, T]]), in_=PW_)
        nc.vector.reciprocal(out=_ap(IPW[:], 1, [[2, T]]), in_=PH_)

        # broadcast-AP helpers over [q,(t,j)]
        def bj(ap2d):   # [128,T] plane -> [128,T,J] broadcasting over j
            return bass.AP(tensor=ap2d.tensor, offset=ap2d.offset,
                           ap=[ap2d.ap[0], list(ap2d.ap[1]), [0, J]])

        def bt(ap2d, step=1):  # [128,J] plane -> [128,T,J] broadcasting over t
            return bass.AP(tensor=ap2d.tensor, offset=ap2d.offset,
                           ap=[ap2d.ap[0], [0, T], [step, J]])

        accs = {}   # name -> [128,1] ap (running) or None

        def acc_add(name, col_ap):
            if name not in accs:
                accs[name] = col_ap
            else:
                nt = accp.tile([128, 1], F32, tag="acc_" + name)
                nc.vector.tensor_tensor(out=nt[:], in0=accs[name], in1=col_ap, op=ALU.add)
                accs[name] = nt[:]

        aconf = [None]
        mine_st = []

        for b in range(NB):
            # ---------------- targets ----------------
            t_row = tp.tile([1, J * 6], F32, tag="trow_%d" % b)
            nc.sync.dma_start(out=t_row[:], in_=bass.AP(tensor=tgt, offset=b * J * 6,
                              ap=[[0, 1], [1, J * 6]]))
            TB = tp.tile([128, J * 6], F32, tag="tb_%d" % b)
            nc.gpsimd.partition_broadcast(TB[:], t_row[:])
            TBP2 = tp.tile([128, J * 6], F32, tag="tbp2_%d" % b)
            nc.vector.tensor_scalar(out=TBP2[:], in0=TB[:], scalar1=2.0, scalar2=None, op0=ALU.add)
            TGJ = tp.tile([J, 6], F32, tag="tgj_%d" % b)
            nc.sync.dma_start(out=TGJ[:], in_=bass.AP(tensor=tgt, offset=b * J * 6,
                              ap=[[6, J], [1, 6]]))
            tb = lambda k: _ap(TB[:], k, [[6, J]])     # [128,J] col view
            tb2 = lambda k: _ap(TBP2[:], k, [[6, J]])
            AT = tp.tile([128, J], F32, tag="areat_%d" % b)
            e1 = sm.tile([128, J], F32, tag="e1")
            e2 = sm.tile([128, J], F32, tag="e2")
            nc.vector.tensor_tensor(out=e1[:], in0=tb(2), in1=tb(0), op=ALU.subtract)
            nc.vector.tensor_tensor(out=e2[:], in0=tb(3), in1=tb(1), op=ALU.subtract)
            nc.vector.tensor_tensor(out=AT[:], in0=e1[:], in1=e2[:], op=ALU.mult)

            # ---------------- pairwise [128, T*J] ----------------
            def big(tag):
                t = bp_.tile([128, T * J], F32, tag=tag)
                return t, bass.AP(tensor=t[:].tensor, offset=t[:].offset,
                                  ap=[t[:].ap[0], [J, T], [1, J]])
            A, Av = big("bigA")
            Bt, Bv = big("bigB")
            nc.vector.tensor_tensor(out=Av, in0=bj(pd(0)), in1=bt(tb(0), 6), op=ALU.max)
            nc.vector.tensor_tensor(out=Bv, in0=bj(pd(1)), in1=bt(tb(2), 6), op=ALU.min)
            Ct, Cv = big("bigC")
            nc.gpsimd.tensor_tensor(out=Cv, in0=Bv, in1=Av, op=ALU.subtract)   # wx
            A, Av = big("bigA")
            nc.scalar.activation(out=Av, in_=Cv, func=ACTF.Relu)               # wx+
            Bt, Bv = big("bigB")
            Ct, Cv = big("bigC")
            nc.vector.tensor_tensor(out=Bv, in0=bj(pd(2)), in1=bt(tb(1), 6), op=ALU.max)
            nc.vector.tensor_tensor(out=Cv, in0=bj(pd(3)), in1=bt(tb(3), 6), op=ALU.min)
            Dt, Dv = big("bigD")
            nc.gpsimd.tensor_tensor(out=Dv, in0=Cv, in1=Bv, op=ALU.subtract)   # wy
            Bt, Bv = big("bigB")
            nc.scalar.activation(out=Bv, in_=Dv, func=ACTF.Relu)               # wy+
            Ct, Cv = big("bigC")
            nc.vector.tensor_tensor(out=Cv, in0=Av, in1=Bv, op=ALU.mult)       # inter
            Dt, Dv = big("bigD")
            nc.gpsimd.tensor_tensor(out=Dv, in0=bj(pd(4)), in1=bt(AT[:]), op=ALU.add)  # AS
            # pos3 = (3*inter >= AS), reduce over j
            A, Av = big("bigA")
            nc.vector.scalar_tensor_tensor(out=Av, in0=Cv, scalar=3.0, in1=Dv,
                                           op0=ALU.mult, op1=ALU.is_ge)
            POSQ = pmp.tile([128, T], F32, tag="posq")
            nc.vector.tensor_reduce(out=POSQ[:], in_=Av, axis=AXL.X, op=ALU.max)
            A, Av = big("bigA")
            nc.vector.reciprocal(out=Av, in_=Dv)                               # 1/AS
            Dt, Dv = big("bigD")
            nc.vector.tensor_tensor(out=Dv, in0=Cv, in1=Av, op=ALU.mult)       # R
            A, Av = big("bigA")
            nc.gpsimd.tensor_tensor(out=Av, in0=Dv, in1=bt(EPSJ[:]), op=ALU.add)  # R'
            # --- force-match: vmax per truth, eqp, F2, R'' = max(R', F2)
            MQ = sm.tile([128, J], F32, tag="mq")
            cjt = bass.AP(tensor=A[:].tensor, offset=A[:].offset,
                          ap=[A[:].ap[0], [1, J], [J, T]])
            nc.vector.tensor_reduce(out=MQ[:], in_=cjt, axis=AXL.X, op=ALU.max)
            VMB = sm.tile([128, J], F32, tag="vmb")
            nc.gpsimd.partition_all_reduce(VMB[:], MQ[:], channels=128,
                                           reduce_op=bass_isa.ReduceOp.max)
            Bt, Bv = big("bigB")
            nc.vector.tensor_tensor(out=Bv, in0=Av, in1=bt(VMB[:]), op=ALU.is_ge)  # eqp
            Ct, Cv = big("bigC")
            nc.vector.tensor_tensor(out=Cv, in0=Bv, in1=bt(J2[:]), op=ALU.mult)    # F2
            Bt, Bv = big("bigB")
            nc.vector.tensor_tensor(out=Bv, in0=Av, in1=Cv, op=ALU.max)            # R''
            # per-prior max + EQ
            MR = pmp.tile([128, T], F32, tag="mr")
            nc.vector.tensor_reduce(out=MR[:], in_=Bv, axis=AXL.X, op=ALU.max)
            Ct, Cv = big("bigC")
            nc.vector.tensor_tensor(out=Cv, in0=Bv, in1=bj(MR[:]), op=ALU.is_ge)   # EQ
            fm = sm.tile([128, T], F32, tag="fm")
            nc.vector.tensor_scalar(out=fm[:], in0=MR[:], scalar1=1.5, scalar2=None, op0=ALU.is_ge)
            nc.vector.tensor_tensor(out=POSQ[:], in0=POSQ[:], in1=fm[:], op=ALU.max)
            # payload gathers: w, x1, y1, x2, y2  (value+2 trick)
            PAY = pmp.tile([128, T, 5], F32, tag="pay")
            cols = [5, 0, 1, 2, 3]
            for vi, k in enumerate(cols):
                Dt, Dv = big("bigD")
                eng = nc.gpsimd if vi in (1, 3) else nc.vector
                eng.tensor_tensor(out=Dv, in0=Cv, in1=bt(tb2(k), 6), op=ALU.mult)
                g = sm.tile([128, T], F32, tag="gv")
                nc.vector.tensor_reduce(out=g[:], in_=Dv, axis=AXL.X, op=ALU.max)
                nc.vector.tensor_scalar(out=_ap(PAY[:], vi, [[5, T]]), in0=g[:],
                                        scalar1=2.0, scalar2=None, op0=ALU.subtract)
            W = _ap(PAY[:], 0, [[5, T]])
            MX1, MY1, MX2, MY2 = (_ap(PAY[:], i, [[5, T]]) for i in (1, 2, 3, 4))
            # PM = EQ * posq  (keep for conf stack)
            PMT = pmp.tile([128, T * J], F32, tag="pmt")
            PMv = bass.AP(tensor=PMT[:].tensor, offset=PMT[:].offset,
                          ap=[PMT[:].ap[0], [J, T], [1, J]])
            nc.vector.tensor_tensor(out=PMv, in0=Cv, in1=bj(POSQ[:]), op=ALU.mult)

            # ---------------- conf phase ----------------
            do_conf = PH >= 2
            if do_conf:
                CPS = psp.tile([J, 80], F32, tag="ps_c")
                LSES = pmp.tile([128, T], F32, tag="lses")
                for tci in range(TC):
                    CH = cfp.tile([128, TCH * C], F32, tag="ch")
                    nc.sync.dma_start(out=CH[:], in_=bass.AP(
                        tensor=conf, offset=b * P * C + tci * TCH * 128 * C,
                        ap=[[C, 128], [128 * C, TCH], [1, C]]))
                    EX = cfp.tile([128, TCH * C], F32, tag="ex")
                    nc.scalar.activation(out=EX[:], in_=CH[:], func=ACTF.Exp)
                    exv = bass.AP(tensor=EX[:].tensor, offset=EX[:].offset,
                                  ap=[EX[:].ap[0], [C, TCH], [1, C]])
                    nc.vector.tensor_reduce(out=LSES[:, tci * TCH:(tci + 1) * TCH],
                                            in_=exv, axis=AXL.X, op=ALU.add)
                    for ti in range(TCH):
                        tg = tci * TCH + ti
                        nc.tensor.matmul(CPS[:], PMT[:, tg * J:(tg + 1) * J],
                                         CH[:, ti * C:(ti + 1) * C],
                                         start=(tg == 0), stop=(tg == T - 1))
                LSE = pmp.tile([128, T], F32, tag="lse")
                nc.scalar.activation(out=LSE[:], in_=LSES[:], func=ACTF.Ln)
                # conf-gather term: sum_j w_j * CPS[j, lab_j-1]
                lab1 = sm.tile([J, 1], F32, tag="lab1")
                nc.vector.tensor_scalar(out=lab1[:], in0=TGJ[:, 4:5], scalar1=1.0,
                                        scalar2=None, op0=ALU.subtract)
                OH = sm.tile([J, 80], F32, tag="oh")
                nc.vector.tensor_scalar(out=OH[:], in0=IOTA80[:J, :], scalar1=lab1[:],
                                        scalar2=None, op0=ALU.is_equal)
                sc1 = sm.tile([J, 80], F32, tag="sc1")
                sj = sm.tile([J, 1], F32, tag="sj")
                nc.vector.tensor_tensor(out=sc1[:], in0=OH[:], in1=CPS[:], op=ALU.mult)
                nc.vector.tensor_reduce(out=sj[:], in_=sc1[:], axis=AXL.X, op=ALU.add)
                cj = accp.tile([J, 1], F32, tag="cfj_%d" % b)
                nc.vector.tensor_tensor(out=cj[:], in0=sj[:], in1=TGJ[:, 5:6], op=ALU.mult)
                if aconf[0] is None:
                    aconf[0] = cj[:]
                else:
                    nj = accp.tile([J, 1], F32, tag="cfj2_%d" % b)
                    nc.vector.tensor_tensor(out=nj[:], in0=aconf[0], in1=cj[:], op=ALU.add)
                    aconf[0] = nj[:]

            # ---------------- obj / pw / mine prep ----------------
            OBJ = pmp.tile([128, T * 2], F32, tag="objt")
            nc.sync.dma_start(out=OBJ[:], in_=bass.AP(tensor=obj, offset=b * P * 2,
                              ap=[[2, 128], [256, T], [1, 2]]))
            O0 = _ap(OBJ[:], 0, [[2, T]])
            O1 = _ap(OBJ[:], 1, [[2, T]])
            dm = sm.tile([128, T], F32, tag="dm")
            nc.vector.tensor_tensor(out=dm[:], in0=O1, in1=O0, op=ALU.subtract)
            sp = pmp.tile([128, T], F32, tag="sp")
            nc.scalar.activation(out=sp[:], in_=dm[:], func=ACTF.Exp)
            nc.scalar.activation(out=sp[:], in_=sp[:], func=ACTF.Ln, bias=1.0)
            ceo = sm.tile([128, T], F32, tag="ceo")
            nc.vector.tensor_tensor(out=ceo[:], in0=POSQ[:], in1=dm[:], op=ALU.mult)
            nc.vector.tensor_tensor(out=ceo[:], in0=sp[:], in1=ceo[:], op=ALU.subtract)
            PWt = pmp.tile([128, T], F32, tag="pw")
            nc.vector.tensor_tensor(out=PWt[:], in0=W, in1=POSQ[:], op=ALU.mult)
            MINE = pmp.tile([128, T], F32, tag="mine_%d" % b)
            negq = sm.tile([128, T], F32, tag="negq")
            nc.vector.tensor_scalar(out=negq[:], in0=POSQ[:], scalar1=-1.0, scalar2=1.0,
                                    op0=ALU.mult, op1=ALU.add)
            nc.vector.tensor_tensor(out=MINE[:], in0=sp[:], in1=negq[:], op=ALU.mult)
            MW = pmp.tile([128, T], F32, tag="mw_%d" % b)
            nc.vector.tensor_tensor(out=MW[:], in0=MINE[:], in1=W, op=ALU.mult)
            # accumulations
            scr = sm.tile([128, T], F32, tag="scr")
            c1 = accp.tile([128, 1], F32, tag="c1_%d" % b)
            nc.vector.tensor_tensor(out=scr[:], in0=PWt[:], in1=ceo[:], op=ALU.mult)
            nc.vector.tensor_reduce(out=c1[:], in_=scr[:], axis=AXL.X, op=ALU.add)
            acc_add("ceo", c1[:])
            if do_conf:
                c2 = accp.tile([128, 1], F32, tag="c2_%d" % b)
                nc.vector.tensor_tensor(out=scr[:], in0=PWt[:], in1=LSE[:], op=ALU.mult)
                nc.vector.tensor_reduce(out=c2[:], in_=scr[:], axis=AXL.X, op=ALU.add)
                acc_add("lse", c2[:])
            c3 = accp.tile([128, 1], F32, tag="c3_%d" % b)
            nc.vector.tensor_reduce(out=c3[:], in_=PWt[:], axis=AXL.X, op=ALU.add)
            acc_add("n", c3[:])
            mine_st.append((MINE, MW, c3))

            # ---------------- smooth-L1 ----------------
            do_sl1 = PH != 0 and PH != 4
            LOC = pmp.tile([128, T * 4], F32, tag="loct")
            nc.sync.dma_start(out=LOC[:], in_=bass.AP(tensor=loc, offset=b * P * 4,
                              ap=[[4, 128], [512, T], [1, 4]]))
            if do_sl1:
                SL = sm.tile([128, T], F32, tag="sl")
                u1 = sm.tile([128, T], F32, tag="u1")
                u2 = sm.tile([128, T], F32, tag="u2")
                u3 = sm.tile([128, T], F32, tag="u3")
                for ci in range(4):
                    lc = _ap(LOC[:], ci, [[4, T]])
                    if ci < 2:
                        m1, m2 = (MX1, MX2) if ci == 0 else (MY1, MY2)
                        pc = PCX if ci == 0 else PCY
                        idp = pd(5) if ci == 0 else pd(6)
                        nc.vector.tensor_tensor(out=u1[:], in0=m1, in1=m2, op=ALU.add)
                        nc.vector.scalar_tensor_tensor(out=u2[:], in0=u1[:], scalar=0.5,
                                                       in1=pc, op0=ALU.mult, op1=ALU.subtract)
                        nc.vector.tensor_tensor(out=u1[:], in0=u2[:], in1=idp, op=ALU.mult)
                    else:
                        m1, m2 = (MX1, MX2) if ci == 2 else (MY1, MY2)
                        ip = _ap(IPW[:], 0 if ci == 2 else 1, [[2, T]])
                        nc.vector.tensor_tensor(out=u1[:], in0=m2, in1=m1, op=ALU.subtract)
                        nc.vector.tensor_tensor(out=u2[:], in0=u1[:], in1=ip, op=ALU.mult)
                        nc.scalar.activation(out=u3[:], in_=u2[:], func=ACTF.Ln)
                        nc.vector.tensor_scalar(out=u1[:], in0=u3[:],
                                                scalar1=float(np.float32(1.0) / np.float32(0.2)),
                                                scalar2=None, op0=ALU.mult)
                    nc.vector.tensor_tensor(out=u2[:], in0=lc, in1=u1[:], op=ALU.subtract)
                    nc.scalar.activation(out=u3[:], in_=u2[:], func=ACTF.Abs)
                    nc.vector.tensor_scalar(out=u1[:], in0=u3[:], scalar1=1.0, scalar2=None,
                                            op0=ALU.min)
                    nc.vector.scalar_tensor_tensor(out=u2[:], in0=u1[:], scalar=-0.5,
                                                   in1=u3[:], op0=ALU.mult, op1=ALU.add)
                    if ci == 0:
                        nc.vector.tensor_tensor(out=SL[:], in0=u1[:], in1=u2[:], op=ALU.mult)
                    else:
                        nc.vector.tensor_tensor(out=u3[:], in0=u1[:], in1=u2[:], op=ALU.mult)
                        nc.vector.tensor_tensor(out=SL[:], in0=SL[:], in1=u3[:], op=ALU.add)
                c4 = accp.tile([128, 1], F32, tag="c4_%d" % b)
                scr4 = sm.tile([128, T], F32, tag="scr4")
                nc.vector.tensor_tensor(out=scr4[:], in0=PWt[:], in1=SL[:], op=ALU.mult)
                nc.vector.tensor_reduce(out=c4[:], in_=scr4[:], axis=AXL.X, op=ALU.add)
                acc_add("sl1", c4[:])

        # ---------------- mining (batched binary search) ----------------
        NP4 = accp.tile([128, NB], F32, tag="np4")
        for b in range(NB):
            nc.vector.tensor_copy(out=NP4[:, b:b + 1], in_=mine_st[b][2][:])
        NPS = psp.tile([128, NB], F32, tag="ps_np")
        nc.tensor.matmul(NPS[:], ONES[:], NP4[:], start=True, stop=True)
        NPT = accp.tile([128, NB], F32, tag="npt")
        nc.scalar.copy(out=NPT[:], in_=NPS[:])
        NPI = accp.tile([128, NB], mybir.dt.int32, tag="npi")
        nc.vector.tensor_copy(out=NPI[:], in_=NPT[:])
        FR = accp.tile([128, NB], F32, tag="fr")
        nc.vector.tensor_copy(out=FR[:], in_=NPI[:])
        GG = accp.tile([128, NB], F32, tag="gg")
        nc.vector.tensor_tensor(out=GG[:], in0=FR[:], in1=NPT[:], op=ALU.is_gt)
        K4 = accp.tile([128, NB], F32, tag="k4")
        nc.vector.tensor_tensor(out=K4[:], in0=FR[:], in1=GG[:], op=ALU.subtract)
        nc.vector.tensor_scalar(out=K4[:], in0=K4[:], scalar1=3.0, scalar2=None, op0=ALU.mult)
        if PH not in (0, 3):
            LO = accp.tile([128, NB], F32, tag="lo4")
            HI = accp.tile([128, NB], F32, tag="hi4")
            nc.vector.memset(LO[:], 0.0)
            nc.vector.memset(HI[:], 16.0)
            MID = accp.tile([128, NB], F32, tag="mid4")
            CNT = accp.tile([128, NB], F32, tag="cnt4")
            for it in range(MINE_ITERS):
                nc.vector.tensor_tensor(out=MID[:], in0=LO[:], in1=HI[:], op=ALU.add)
                nc.vector.tensor_scalar(out=MID[:], in0=MID[:], scalar1=0.5, scalar2=None,
                                        op0=ALU.mult)
                for b in range(NB):
                    scx = sm.tile([128, T], F32, tag="scx")
                    nc.vector.tensor_scalar(out=scx[:], in0=mine_st[b][0][:],
                                            scalar1=MID[:, b:b + 1], scalar2=None,
                                            op0=ALU.is_gt)
                    nc.vector.tensor_reduce(out=CNT[:, b:b + 1], in_=scx[:],
                                            axis=AXL.X, op=ALU.add)
                CPSUM = psp.tile([128, NB], F32, tag="ps_cnt")
                nc.tensor.matmul(CPSUM[:], ONES[:], CNT[:], start=True, stop=True)
                GE = accp.tile([128, NB], F32, tag="ge4")
                nc.scalar.copy(out=GE[:], in_=CPSUM[:])
                nc.vector.tensor_tensor(out=GE[:], in0=GE[:], in1=K4[:], op=ALU.is_ge)
                d1 = accp.tile([128, NB], F32, tag="d1")
                nc.vector.tensor_tensor(out=d1[:], in0=MID[:], in1=LO[:], op=ALU.subtract)
                nc.vector.tensor_tensor(out=d1[:], in0=GE[:], in1=d1[:], op=ALU.mult)
                nc.vector.tensor_tensor(out=LO[:], in0=LO[:], in1=d1[:], op=ALU.add)
                nc.vector.tensor_tensor(out=d1[:], in0=HI[:], in1=MID[:], op=ALU.subtract)
                nc.vector.tensor_tensor(out=d1[:], in0=GE[:], in1=d1[:], op=ALU.mult)
                nc.vector.tensor_tensor(out=HI[:], in0=MID[:], in1=d1[:], op=ALU.add)
            for b in range(NB):
                scx = sm.tile([128, T], F32, tag="scx")
                c5 = accp.tile([128, 1], F32, tag="c5_%d" % b)
                nc.vector.scalar_tensor_tensor(out=scx[:], in0=mine_st[b][0][:],
                                               scalar=LO[:, b:b + 1], in1=mine_st[b][1][:],
                                               op0=ALU.is_gt, op1=ALU.mult, accum_out=c5[:])
                acc_add("neg", c5[:])

        # ---------------- final assembly ----------------
        FIN = accp.tile([128, 8], F32, tag="fin")
        nc.vector.memset(FIN[:], 0.0)
        for i, nm in enumerate(["sl1", "lse", "ceo", "neg", "n"]):
            if nm in accs:
                nc.vector.tensor_copy(out=FIN[:, i:i + 1], in_=accs[nm])
        if aconf[0] is not None:
            nc.vector.tensor_copy(out=FIN[0:J, 5:6], in_=aconf[0])
        OPS = psp.tile([1, 8], F32, tag="ps_out")
        nc.tensor.matmul(OPS[:], ONES[:, 0:1], FIN[:], start=True, stop=True)
        OUTT = accp.tile([1, 8], F32, tag="outt")
        nc.scalar.copy(out=OUTT[:], in_=OPS[:])
        nc.sync.dma_start(out=bass.AP(tensor=out, offset=0, ap=[[8, 1], [1, 8]]),
                          in_=OUTT[:])
        for pl in (psp, cfp, accp, sm, pmp, bp_, tp, cp):
            pl.release()
    nc.compile()
    return nc


def kernel(loc_data, conf_data, obj_data, priors, targets, trace=False):
    if "nc" not in _CACHE:
        _CACHE["nc"] = build_nc()
    nc = _CACHE["nc"]
    in_maps = []
    for c in range(NCORES):
        s = slice(c * NB, (c + 1) * NB)
        in_maps.append({
            "loc": np.ascontiguousarray(loc_data[s], dtype=np.float32),
            "conf": np.ascontiguousarray(conf_data[s], dtype=np.float32),
            "obj": np.ascontiguousarray(obj_data[s], dtype=np.float32),
            "tgt": np.ascontiguousarray(targets[s], dtype=np.float32),
            "pri": np.ascontiguousarray(priors, dtype=np.float32),
        })
    res = run_bass_kernel_spmd(nc, in_maps, core_ids=list(range(NCORES)), trace=trace)
    tot = np.zeros(8, dtype=np.float64)
    for r in res.results:
        tot += r["out"][0].astype(np.float64)
    sl1, lse, ceo, neg, n, cterm = tot[0], tot[1], tot[2], tot[3], tot[4], tot[5]
    n32 = np.float32(n)
    loss_l = np.float32(sl1) / n32
    loss_c = np.float32(lse + ceo - cterm + neg) / n32
    loss_o = np.float32(ceo + neg) / n32
    if trace:
        kernel.last_exec_ns = res.exec_time_ns
    return (np.float32(loss_l), np.float32(loss_c), np.float32(loss_o))

